# revision 19
# baseline (speedup 1.0000x reference)
"""3-layer GAT on 8 Trainium2 NeuronCores — v2.

Strategy (edge-parallel by destination), changes vs v1:
- exp(lrelu(el+er)) = max(exp(el)exp(er), exp(.2el)exp(.2er)): the table
  stores A=exp(el), C=exp(.2el) bf16 (no f32 logits, no Exp in edge phase);
  per-dst B=exp(er), D=exp(.2er) live in SBUF and are expanded per edge by
  the ohT matmul.  p = max(A*B, C*D) on DVE.
- One-hot matrices are fp8e4 (0/1 exact; matmul allows fp8 lhsT with bf16
  rhs), generated once on device, staged in DRAM, streamed per tile-layer:
  halves one-hot HBM traffic ~4x vs v1 bf16 oh+ohT.
- PSUM->SBUF casts run on the Scalar (ACT) engine, freeing DVE.
- Next-layer table build is fused into the edge-phase tile loop, so the
  AllGather launches immediately when the last tile finishes.
"""

import numpy as np
import ml_dtypes

N_NODES = 50000
N_EDGES = 800000
IN_FEATS = 128
HID = 32
HEADS = 8
OUT_FEATS = 32
NEG_SLOPE = 0.2

NC_N = 8                 # cores
NPC = N_NODES // NC_N    # real nodes per core (6250)
NT = 49                  # dst tiles per core
SLOTS = NT * 128         # 6272 slots per core
HALF = 4 * SLOTS         # 25088 table rows per half
VTOT = NC_N * SLOTS      # 50176 table rows

BF16 = ml_dtypes.bfloat16

_CACHE = {}
_last_in_maps = None


# ----------------------------------------------------------------------------
# Host-side graph preparation (same as v1)
# ----------------------------------------------------------------------------

def _prep_graph(src, dst):
    src = np.asarray(src).astype(np.int64)
    dst = np.asarray(dst).astype(np.int64)

    ecore = dst // NPC

    slot_g = np.zeros(N_NODES, dtype=np.int64)
    degA = np.zeros(N_NODES, dtype=np.int64)
    degB = np.zeros(N_NODES, dtype=np.int64)
    half_e = (src >= 4 * NPC).astype(np.int64)
    np.add.at(degA, dst[half_e == 0], 1)
    np.add.at(degB, dst[half_e == 1], 1)

    for k in range(NC_N):
        lo, hi = k * NPC, (k + 1) * NPC
        nodes = np.arange(lo, hi)
        d = degA[lo:hi] + degB[lo:hi]
        order = np.argsort(-d, kind="stable")
        loads = np.zeros(NT, dtype=np.int64)
        counts = np.zeros(NT, dtype=np.int64)
        tile_of = np.zeros(NPC, dtype=np.int64)
        pos_of = np.zeros(NPC, dtype=np.int64)
        for i in order:
            t = np.argmin(np.where(counts < 128, loads, np.iinfo(np.int64).max))
            tile_of[i] = t
            pos_of[i] = counts[t]
            counts[t] += 1
            loads[t] += d[i]
        slot_g[nodes] = tile_of * 128 + pos_of

    srcslot = (src // NPC) * SLOTS + slot_g[src]
    dslot = slot_g[dst]
    dtile = dslot // 128
    dstl = dslot % 128

    key = (ecore * NT + dtile) * 2 + half_e
    order = np.argsort(key, kind="stable")
    key_s = key[order]
    ngroups = NC_N * NT * 2
    counts = np.bincount(key_s, minlength=ngroups)
    starts = np.concatenate([[0], np.cumsum(counts)[:-1]])
    j_within = np.arange(len(src)) - starts[key_s]

    CPH = int(np.ceil(counts.max() / 128))
    CAP = CPH * 128

    gidx = np.zeros((NC_N, NT, 2, CAP), dtype=np.int16)
    dstl_a = np.full((NC_N, NT, 2, CAP), -1.0, dtype=np.float32)

    ks = key_s
    gidx[ks // (NT * 2), (ks // 2) % NT, ks % 2, j_within] = (
        srcslot[order] - (ks % 2) * HALF
    ).astype(np.int16)
    dstl_a[ks // (NT * 2), (ks // 2) % NT, ks % 2, j_within] = dstl[order]

    CPT = 2 * CPH
    TSLOT = CPT * 128
    idxA, idxB, dstlT, dstlF = [], [], [], []
    for k in range(NC_N):
        ia = gidx[k, :, 0, :].reshape(-1)
        ib = gidx[k, :, 1, :].reshape(-1)
        wrapA = np.tile(ia.reshape(-1, 16).T, (8, 1))
        wrapB = np.tile(ib.reshape(-1, 16).T, (8, 1))
        idxA.append(np.ascontiguousarray(wrapA))
        idxB.append(np.ascontiguousarray(wrapB))
        dT2 = dstl_a[k].reshape(NT, 2, CPH, 128)     # [t, h, c_h, p]
        dT2 = dT2.reshape(NT, CPT, 128)              # [t, c, p]
        dstlT.append(np.ascontiguousarray(
            dT2.transpose(2, 0, 1).reshape(128, NT * CPT).astype(BF16)))
        dstlF.append(np.ascontiguousarray(dT2.reshape(NT, TSLOT).astype(BF16)))

    return {
        "CPH": CPH,
        "slot_g": slot_g,
        "idxA": idxA, "idxB": idxB,
        "dstlT": dstlT, "dstlF": dstlF,
    }


def _fold_w(W, al, ar):
    """Wc = [Wl | W | Wr]: el = h@Wl, ft = h@W, er = h@Wr."""
    Din = W.shape[0]
    H, C = al.shape
    W3 = W.reshape(Din, H, C)
    Wl = np.einsum("dhc,hc->dh", W3, al)
    Wr = np.einsum("dhc,hc->dh", W3, ar)
    return np.concatenate([Wl, W, Wr], axis=1).astype(BF16)  # [Din, H + H*C + H]


# ----------------------------------------------------------------------------
# Device program
# ----------------------------------------------------------------------------

def _build_program(CPH, stages=None):
    import concourse.bass as bass
    import concourse.mybir as mybir
    import concourse.tile as tile
    from concourse import bacc
    from concourse.masks import make_identity

    f32 = mybir.dt.float32
    bf16 = mybir.dt.bfloat16
    fp8 = mybir.dt.float8e4
    i16 = mybir.dt.int16
    Alu = mybir.AluOpType
    Act = mybir.ActivationFunctionType

    CPT = 2 * CPH
    TSLOT = CPT * 128
    ICOL = NT * CPH * 8          # idx cols per half: NT*CPH*128/16

    nc = bacc.Bacc("TRN2", target_bir_lowering=False, debug=False,
                   num_devices=NC_N, num_swdge_queues=4)

    # ---- I/O ----
    xT = nc.dram_tensor("xT", [128, SLOTS], bf16, kind="ExternalInput")
    wc0 = nc.dram_tensor("wc0", [128, 272], bf16, kind="ExternalInput")
    wc1 = nc.dram_tensor("wc1", [256, 272], bf16, kind="ExternalInput")
    wc2 = nc.dram_tensor("wc2", [256, 34], bf16, kind="ExternalInput")
    idxA = nc.dram_tensor("idxA", [128, ICOL], i16, kind="ExternalInput")
    idxB = nc.dram_tensor("idxB", [128, ICOL], i16, kind="ExternalInput")
    dstlT = nc.dram_tensor("dstlT", [128, NT * CPT], bf16, kind="ExternalInput")
    dstlF = nc.dram_tensor("dstlF", [NT, TSLOT], bf16, kind="ExternalInput")
    iota128 = nc.dram_tensor("iota128", [128, 128], bf16, kind="ExternalInput")
    iotaPb = nc.dram_tensor("iotaPb", [128, 1], bf16, kind="ExternalInput")
    ones128 = nc.dram_tensor("ones128", [128, 128], bf16, kind="ExternalInput")
    out_d = nc.dram_tensor("out", [SLOTS, 32], f32, kind="ExternalOutput")

    # ---- internal DRAM ----
    ohC = nc.dram_tensor("ohC", [128, NT, 2 * TSLOT], fp8)
    T0s = nc.dram_tensor("T0s", [SLOTS, 512], fp8)
    T1s = nc.dram_tensor("T1s", [SLOTS, 512], fp8)
    T2s = nc.dram_tensor("T2s", [SLOTS, 128], bf16)
    T0f = nc.dram_tensor("T0f", [VTOT, 512], fp8, addr_space="Shared")
    T1f = nc.dram_tensor("T1f", [VTOT, 512], fp8, addr_space="Shared")
    T2f = nc.dram_tensor("T2f", [VTOT, 128], bf16, addr_space="Shared")

    def bc(ap, dims):
        return bass.AP(ap.tensor, ap.offset, [ap.ap[0]] + dims)

    with tile.TileContext(nc) as tc:
        import contextlib
        ctx = contextlib.ExitStack()
        with ctx:
            consts = ctx.enter_context(tc.tile_pool(name="consts", bufs=1))
            persist = ctx.enter_context(tc.tile_pool(name="persist", bufs=1))
            gpool = ctx.enter_context(tc.tile_pool(name="gather", bufs=3))
            mpool = ctx.enter_context(tc.tile_pool(name="msg", bufs=2))
            opool = ctx.enter_context(tc.tile_pool(name="oneh", bufs=3))
            spool = ctx.enter_context(tc.tile_pool(name="small", bufs=3))
            tpool = ctx.enter_context(tc.tile_pool(name="tbuild", bufs=2))
            pp_tb = ctx.enter_context(tc.tile_pool(name="ps_tb", bufs=2, space="PSUM"))
            pp_erd = ctx.enter_context(tc.tile_pool(name="ps_erd", bufs=2, space="PSUM"))
            pp_agg = ctx.enter_context(tc.tile_pool(name="ps_agg", bufs=2, space="PSUM"))
            pp_rep = ctx.enter_context(tc.tile_pool(name="ps_rep", bufs=1, space="PSUM"))
            pp_tp = ctx.enter_context(tc.tile_pool(name="ps_tp", bufs=1, space="PSUM"))

            # ---- load constants ----
            iota128_sb = consts.tile([128, 128], bf16)
            nc.sync.dma_start(out=iota128_sb[:], in_=iota128[:, :])
            iotaPb_sb = consts.tile([128, 1], bf16)
            nc.sync.dma_start(out=iotaPb_sb[:], in_=iotaPb[:, :])
            ones128_sb = consts.tile([128, 128], bf16)
            nc.sync.dma_start(out=ones128_sb[:], in_=ones128[:, :])
            ident = consts.tile([128, 128], bf16)
            make_identity(nc, ident[:])
            zer_bf = consts.tile([128, 1], bf16)
            nc.gpsimd.memset(zer_bf[:], 0)
            zer_f32 = consts.tile([128, 1], f32)
            nc.gpsimd.memset(zer_f32[:], 0)
            eps_f32 = consts.tile([128, 1], f32)
            nc.gpsimd.memset(eps_f32[:], 1e-30)

            idxA_sb = persist.tile([128, ICOL], i16)
            nc.sync.dma_start(out=idxA_sb[:], in_=idxA[:, :])
            idxB_sb = persist.tile([128, ICOL], i16)
            nc.sync.dma_start(out=idxB_sb[:], in_=idxB[:, :])
            dstlT_sb = persist.tile([128, NT * CPT], bf16)
            nc.sync.dma_start(out=dstlT_sb[:], in_=dstlT[:, :])
            xT_sb = persist.tile([128, SLOTS], bf16)
            nc.sync.dma_start(out=xT_sb[:], in_=xT[:, :])
            wc0_sb = persist.tile([128, 272], bf16)
            nc.sync.dma_start(out=wc0_sb[:], in_=wc0[:, :])
            wc1_sb = persist.tile([128, 2, 272], bf16)
            nc.sync.dma_start(out=wc1_sb[:, 0, :], in_=wc1[0:128, :])
            nc.sync.dma_start(out=wc1_sb[:, 1, :], in_=wc1[128:256, :])
            wc2_sb = persist.tile([128, 2, 34], bf16)
            nc.sync.dma_start(out=wc2_sb[:, 0, :], in_=wc2[0:128, :])
            nc.sync.dma_start(out=wc2_sb[:, 1, :], in_=wc2[128:256, :])

            # per-layer dst-side factors: [B=exp(er) | D=exp(.2 er)]
            er0_sb = persist.tile([128, NT, 16], bf16, tag="er0")
            er1_sb = persist.tile([128, NT, 16], bf16, tag="er1")
            er2_sb = persist.tile([128, NT, 2], bf16, tag="er2")
            hT_sb = persist.tile([128, 2, SLOTS], bf16, tag="hT")

            qn = [0]

            def next_q():
                qn[0] = (qn[0] + 1) % 4
                return qn[0]

            gsem = [nc.alloc_semaphore(f"gsem{q}") for q in range(4)]

            # ---------------- one-hot build (once, fp8) ----------------
            def build_onehots():
                for t in range(NT):
                    oh = opool.tile([128, CPT, 128], fp8, tag="ohi")
                    dT = dstlT_sb[:, t * CPT:(t + 1) * CPT]
                    nc.vector.tensor_tensor(
                        out=oh[:],
                        in0=bc(dT, [[dT.ap[1][0], CPT], [0, 128]]),
                        in1=bc(iota128_sb[:], [[0, CPT], [1, 128]]),
                        op=Alu.is_equal)
                    nc.sync.dma_start(
                        out=ohC[:, t, 0:TSLOT],
                        in_=oh[:].rearrange("p c e -> p (c e)"))
                    QS = 512
                    ohT = opool.tile([128, CPT, 128], fp8, tag="ohTi")
                    stage = spool.tile([1, TSLOT], bf16, tag="stage")
                    nc.sync.dma_start(out=stage[:], in_=dstlF[t:t + 1, :])
                    ohT_flat = ohT[:].rearrange("p c e -> p (c e)")
                    for s0 in range(0, TSLOT, QS):
                        s1 = min(s0 + QS, TSLOT)
                        rep = pp_rep.tile([128, QS], f32, tag="rep", space="PSUM")
                        nc.tensor.matmul(out=rep[:, 0:s1 - s0],
                                         lhsT=ones128_sb[0:1, :],
                                         rhs=stage[:, s0:s1],
                                         start=True, stop=True)
                        nc.vector.tensor_tensor(
                            out=ohT_flat[:, s0:s1],
                            in0=bc(iotaPb_sb[:, 0:1], [[0, s1 - s0]]),
                            in1=rep[:, 0:s1 - s0],
                            op=Alu.is_equal)
                    nc.sync.dma_start(
                        out=ohC[:, t, TSLOT:2 * TSLOT],
                        in_=ohT_flat)

            # ---------------- table build ----------------
            # ps cols: [el(H) | ft(FT) | er(H)]
            # L0/L1 row (fp8): [A(H) bf16 | C(H) bf16 | ft(FT) fp8]
            # L2 row (bf16):   [A(1) | C(1) | ft(32)]
            def build_table_tile(layer, t, Ts, er_out, H, FT):
                ps = pp_tb.tile([128, 2 * H + FT], f32, tag="tb_ps", space="PSUM")
                if layer == 0:
                    nc.tensor.matmul(out=ps[:], lhsT=xT_sb[:, t * 128:(t + 1) * 128],
                                     rhs=wc0_sb[:], start=True, stop=True)
                else:
                    w = wc1_sb if layer == 1 else wc2_sb
                    for kb in range(2):
                        nc.tensor.matmul(out=ps[:],
                                         lhsT=hT_sb[:, kb, t * 128:(t + 1) * 128],
                                         rhs=w[:, kb, :],
                                         start=(kb == 0), stop=(kb == 1))
                if FT == 256:
                    tsb = tpool.tile([128, 288], fp8, tag="tsb")
                    acA = tsb[:, 0:2 * H].bitcast(bf16)
                    acC = tsb[:, 2 * H:4 * H].bitcast(bf16)
                    ftv = tsb[:, 4 * H:4 * H + FT]
                    nwr = 288
                else:
                    tsb = tpool.tile([128, 34], bf16, tag="tsb2")
                    acA = tsb[:, 0:H]
                    acC = tsb[:, H:2 * H]
                    ftv = tsb[:, 2 * H:2 * H + FT]
                    nwr = 34
                nc.scalar.activation(out=acA, in_=ps[:, 0:H], func=Act.Exp)
                nc.scalar.activation(out=acC, in_=ps[:, 0:H],
                                     func=Act.Exp, scale=NEG_SLOPE)
                nc.scalar.activation(out=ftv, in_=ps[:, H:H + FT], func=Act.Copy)
                nc.scalar.activation(out=er_out[:, t, 0:H], in_=ps[:, H + FT:2 * H + FT],
                                     func=Act.Exp)
                nc.scalar.activation(out=er_out[:, t, H:2 * H], in_=ps[:, H + FT:2 * H + FT],
                                     func=Act.Exp, scale=NEG_SLOPE)
                nc.sync.dma_start(out=Ts[t * 128:(t + 1) * 128, 0:nwr],
                                  in_=tsb[:, 0:nwr])

            def allgather(Ts, Tf, ncol):
                nc.gpsimd.collective_compute(
                    "AllGather", Alu.bypass,
                    replica_groups=[list(range(NC_N))],
                    ins=[Ts[:, :]], outs=[Tf[:, :]],
                )

            # ---------------- edge phase ----------------
            def edge_phase(layer, Tf, er_sb, H, FT):
                ROW = 512 if FT == 256 else 128     # fp8 cols / bf16 cols
                gdt = fp8 if FT == 256 else bf16
                MSGN = FT + H
                ftc0 = 4 * H if FT == 256 else 2 * H
                for t in range(NT):
                    # gathers (A/B halves) into one [128, CPT, ROW] tile,
                    # prepared ahead on Q7 and fired when Tf is ready
                    # Split each half-gather into 3-chunk pieces so one call's
                    # descriptors (384) fit the 1024-desc ring carveout: Q7
                    # emission never stalls on its own drain, and all 4 queues
                    # drain concurrently.
                    g = gpool.tile([128, CPT, ROW], gdt, tag="g")
                    cols = CPH * 8
                    GS = 5                          # chunks per gather call
                    for half, base in ((0, 0), (1, HALF)):
                        for c0 in range(0, CPH, GS):
                            c1 = min(c0 + GS, CPH)
                            idx = idxA_sb if half == 0 else idxB_sb
                            nc.gpsimd.dma_gather(
                                out_ap=g[:, half * CPH + c0:half * CPH + c1, :],
                                in_ap=Tf[base:base + HALF, :],
                                idxs_ap=idx[:, t * cols + c0 * 8:t * cols + c1 * 8],
                                num_idxs=(c1 - c0) * 128,
                                num_idxs_reg=(c1 - c0) * 128,
                                elem_size=ROW, single_packet=False,
                                queue_num=next_q())

                    # stream fp8 one-hots (oh | ohT in one transfer)
                    ohx = opool.tile([128, 2, CPT, 128], fp8, tag="ohx")
                    nc.sync.dma_start(
                        out=ohx[:].rearrange("p a c e -> p (a c e)"),
                        in_=ohC[:, t, :])
                    oh = ohx[:, 0]
                    ohT = ohx[:, 1]

                    # expand dst factors per edge: erd[e, c, 0:H]=B, [H:2H]=D
                    erd = pp_erd.tile([128, CPT, 2 * H], f32, tag="erd",
                                      space="PSUM")
                    for c in range(CPT):
                        nc.tensor.matmul(out=erd[:, c, :],
                                         lhsT=ohT[:, c, :],
                                         rhs=er_sb[:, t, 0:2 * H],
                                         start=True, stop=True)

                    # p = max(A*B, C*D)   [128, CPT, H] bf16
                    if FT == 256:
                        gA = g[:, :, 0:2 * H].bitcast(bf16)
                        gC = g[:, :, 2 * H:4 * H].bitcast(bf16)
                    else:
                        gA = g[:, :, 0:H]
                        gC = g[:, :, H:2 * H]
                    m1 = spool.tile([128, CPT, H], bf16, tag="m1")
                    m2 = spool.tile([128, CPT, H], bf16, tag="m2")
                    nc.vector.tensor_tensor(
                        out=m1[:], in0=gA,
                        in1=erd[:, :, 0:H], op=Alu.mult)
                    nc.vector.tensor_tensor(
                        out=m2[:], in0=gC,
                        in1=erd[:, :, H:2 * H], op=Alu.mult)
                    p = spool.tile([128, CPT, H], bf16, tag="p")
                    nc.vector.tensor_tensor(out=p[:], in0=m1[:], in1=m2[:],
                                            op=Alu.max)

                    # messages: [p*ft | p]
                    msg = mpool.tile([128, CPT, MSGN], bf16, tag="msg")
                    CW = FT // H                    # channels per head
                    nc.vector.tensor_tensor(
                        out=bc(msg[:, :, 0:FT],
                               [[MSGN, CPT], [CW, H], [1, CW]]),
                        in0=bc(g[:, :, ftc0:ftc0 + FT],
                               [[ROW, CPT], [CW, H], [1, CW]]),
                        in1=bc(p[:], [[H, CPT], [1, H], [0, CW]]),
                        op=Alu.mult)
                    nc.gpsimd.tensor_copy(out=msg[:, :, FT:FT + H], in_=p[:, :, :])

                    # segment reduction
                    agg = pp_agg.tile([128, MSGN], f32, tag="agg", space="PSUM")
                    for c in range(CPT):
                        nc.tensor.matmul(out=agg[:], lhsT=oh[:, c, :],
                                         rhs=msg[:, c, :],
                                         start=(c == 0), stop=(c == CPT - 1))

                    # epilogue
                    s_sb = spool.tile([128, H], f32, tag="s")
                    nc.vector.tensor_tensor(out=s_sb[:], in0=agg[:, FT:FT + H],
                                            in1=bc(eps_f32[:, 0:1], [[0, H]]),
                                            op=Alu.add)
                    rs = spool.tile([128, H], f32, tag="rs")
                    nc.vector.reciprocal(out=rs[:], in_=s_sb[:])
                    if layer < 2:
                        h_t = spool.tile([128, 256], bf16, tag="ht")
                        nc.vector.tensor_tensor(
                            out=bc(h_t[:], [[32, H], [1, 32]]),
                            in0=bc(agg[:, 0:FT], [[32, H], [1, 32]]),
                            in1=bc(rs[:], [[1, H], [0, 32]]),
                            op=Alu.mult)
                        nc.vector.tensor_tensor(out=h_t[:], in0=h_t[:],
                                                in1=bc(zer_bf[:, 0:1], [[0, 256]]),
                                                op=Alu.max)
                        for b in range(2):
                            tp = pp_tp.tile([128, 128], bf16, tag="tp",
                                            space="PSUM")
                            nc.tensor.transpose(out=tp[:],
                                                in_=h_t[:, b * 128:(b + 1) * 128],
                                                identity=ident[:])
                            nc.scalar.activation(
                                out=hT_sb[:, b, t * 128:(t + 1) * 128],
                                in_=tp[:], func=Act.Copy)
                        # fused next-layer table build for this tile
                        if layer == 0:
                            build_table_tile(1, t, T1s, er1_sb, 8, 256)
                        else:
                            build_table_tile(2, t, T2s, er2_sb, 1, 32)
                    else:
                        o1 = spool.tile([128, 32], f32, tag="o1")
                        nc.vector.tensor_tensor(
                            out=o1[:], in0=agg[:, 0:32],
                            in1=bc(rs[:, 0:1], [[0, 32]]), op=Alu.mult)
                        nc.vector.tensor_tensor(out=o1[:], in0=o1[:],
                                                in1=bc(zer_f32[:, 0:1], [[0, 32]]),
                                                op=Alu.max)
                        nc.scalar.activation(out=o1[:], in_=o1[:], func=Act.Exp)
                        ssum = spool.tile([128, 1], f32, tag="ssum")
                        nc.vector.tensor_reduce(out=ssum[:], in_=o1[:],
                                                axis=mybir.AxisListType.X,
                                                op=Alu.add)
                        rr = spool.tile([128, 1], f32, tag="rr")
                        nc.vector.reciprocal(out=rr[:], in_=ssum[:])
                        ofin = spool.tile([128, 32], f32, tag="ofin")
                        nc.vector.tensor_tensor(out=ofin[:], in0=o1[:],
                                                in1=bc(rr[:], [[0, 32]]),
                                                op=Alu.mult)
                        nc.sync.dma_start(out=out_d[t * 128:(t + 1) * 128, :],
                                          in_=ofin[:])

            # ================= schedule =================
            all_stages = ["t0", "ag0", "oh", "e0", "ag1", "e1", "ag2", "e2"]
            st = all_stages if stages is None else stages
            if "t0" in st:
                for t in range(NT):
                    build_table_tile(0, t, T0s, er0_sb, 8, 256)
            if "ag0" in st:
                allgather(T0s, T0f, 272)
            if "oh" in st:
                build_onehots()
            if "e0" in st:
                edge_phase(0, T0f, er0_sb, 8, 256)
            if "ag1" in st:
                allgather(T1s, T1f, 272)
            if "e1" in st:
                edge_phase(1, T1f, er1_sb, 8, 256)
            if "ag2" in st:
                allgather(T2s, T2f, 34)
            if "e2" in st:
                edge_phase(2, T2f, er2_sb, 1, 32)

    nc.compile()
    return nc


# ----------------------------------------------------------------------------
# Entry point
# ----------------------------------------------------------------------------

def kernel(x, src, dst, W0, al0, ar0, b0, W1, al1, ar1, b1, W2, al2, ar2, b2):
    from concourse.bass_utils import run_bass_kernel_spmd

    x = np.asarray(x, dtype=np.float32)
    g = _prep_graph(src, dst)
    CPH = g["CPH"]

    key = ("prog", CPH)
    if key not in _CACHE:
        _CACHE[key] = _build_program(CPH)
    nc = _CACHE[key]

    wc0 = _fold_w(np.asarray(W0, np.float32), np.asarray(al0, np.float32),
                  np.asarray(ar0, np.float32))
    wc1 = _fold_w(np.asarray(W1, np.float32), np.asarray(al1, np.float32),
                  np.asarray(ar1, np.float32))
    wc2 = _fold_w(np.asarray(W2, np.float32), np.asarray(al2, np.float32),
                  np.asarray(ar2, np.float32))

    iota128 = np.broadcast_to(np.arange(128, dtype=np.float32), (128, 128)) \
        .astype(BF16)
    iotaPb = np.arange(128, dtype=np.float32).reshape(128, 1).astype(BF16)
    ones128 = np.ones((128, 128), dtype=BF16)

    slot_g = g["slot_g"]
    in_maps = []
    for k in range(NC_N):
        lo, hi = k * NPC, (k + 1) * NPC
        xTk = np.zeros((128, SLOTS), dtype=BF16)
        xTk[:, slot_g[lo:hi]] = x[lo:hi].T.astype(BF16)
        in_maps.append({
            "xT": np.ascontiguousarray(xTk),
            "wc0": wc0, "wc1": wc1, "wc2": wc2,
            "idxA": g["idxA"][k], "idxB": g["idxB"][k],
            "dstlT": g["dstlT"][k], "dstlF": g["dstlF"][k],
            "iota128": iota128, "iotaPb": iotaPb,
            "ones128": ones128,
        })

    global _last_in_maps
    _last_in_maps = in_maps
    res = run_bass_kernel_spmd(nc, in_maps, core_ids=list(range(NC_N)))

    out = np.empty((N_NODES, 32), dtype=np.float32)
    for k in range(NC_N):
        lo, hi = k * NPC, (k + 1) * NPC
        out[lo:hi] = res.results[k]["out"][slot_g[lo:hi]]
    return out


# revision 21
# speedup vs baseline: 1.2371x; 1.2371x over previous
"""3-layer GAT on 8 Trainium2 NeuronCores — v2.

Strategy (edge-parallel by destination), changes vs v1:
- exp(lrelu(el+er)) = max(exp(el)exp(er), exp(.2el)exp(.2er)): the table
  stores A=exp(el), C=exp(.2el) bf16 (no f32 logits, no Exp in edge phase);
  per-dst B=exp(er), D=exp(.2er) live in SBUF and are expanded per edge by
  the ohT matmul.  p = max(A*B, C*D) on DVE.
- One-hot matrices are fp8e4 (0/1 exact; matmul allows fp8 lhsT with bf16
  rhs), generated once on device, staged in DRAM, streamed per tile-layer:
  halves one-hot HBM traffic ~4x vs v1 bf16 oh+ohT.
- PSUM->SBUF casts run on the Scalar (ACT) engine, freeing DVE.
- Next-layer table build is fused into the edge-phase tile loop, so the
  AllGather launches immediately when the last tile finishes.
"""

import numpy as np
import ml_dtypes

N_NODES = 50000
N_EDGES = 800000
IN_FEATS = 128
HID = 32
HEADS = 8
OUT_FEATS = 32
NEG_SLOPE = 0.2

NC_N = 8                 # cores
NPC = N_NODES // NC_N    # real nodes per core (6250)
NT = 49                  # dst tiles per core
SLOTS = NT * 128         # 6272 slots per core
HALF = 4 * SLOTS         # 25088 table rows per half
VTOT = NC_N * SLOTS      # 50176 table rows

BF16 = ml_dtypes.bfloat16

_CACHE = {}
_last_in_maps = None


# ----------------------------------------------------------------------------
# Host-side graph preparation (same as v1)
# ----------------------------------------------------------------------------

def _prep_graph(src, dst):
    src = np.asarray(src).astype(np.int64)
    dst = np.asarray(dst).astype(np.int64)

    ecore = dst // NPC

    slot_g = np.zeros(N_NODES, dtype=np.int64)
    degA = np.zeros(N_NODES, dtype=np.int64)
    degB = np.zeros(N_NODES, dtype=np.int64)
    half_e = (src >= 4 * NPC).astype(np.int64)
    np.add.at(degA, dst[half_e == 0], 1)
    np.add.at(degB, dst[half_e == 1], 1)

    for k in range(NC_N):
        lo, hi = k * NPC, (k + 1) * NPC
        nodes = np.arange(lo, hi)
        d = degA[lo:hi] + degB[lo:hi]
        order = np.argsort(-d, kind="stable")
        loads = np.zeros(NT, dtype=np.int64)
        counts = np.zeros(NT, dtype=np.int64)
        tile_of = np.zeros(NPC, dtype=np.int64)
        pos_of = np.zeros(NPC, dtype=np.int64)
        for i in order:
            t = np.argmin(np.where(counts < 128, loads, np.iinfo(np.int64).max))
            tile_of[i] = t
            pos_of[i] = counts[t]
            counts[t] += 1
            loads[t] += d[i]
        slot_g[nodes] = tile_of * 128 + pos_of

    srcslot = (src // NPC) * SLOTS + slot_g[src]
    dslot = slot_g[dst]
    dtile = dslot // 128
    dstl = dslot % 128

    key = (ecore * NT + dtile) * 2 + half_e
    order = np.argsort(key, kind="stable")
    key_s = key[order]
    ngroups = NC_N * NT * 2
    counts = np.bincount(key_s, minlength=ngroups)
    starts = np.concatenate([[0], np.cumsum(counts)[:-1]])
    j_within = np.arange(len(src)) - starts[key_s]

    CPH = int(np.ceil(counts.max() / 128))
    CAP = CPH * 128

    gidx = np.zeros((NC_N, NT, 2, CAP), dtype=np.int16)
    dstl_a = np.full((NC_N, NT, 2, CAP), -1.0, dtype=np.float32)

    ks = key_s
    gidx[ks // (NT * 2), (ks // 2) % NT, ks % 2, j_within] = (
        srcslot[order] - (ks % 2) * HALF
    ).astype(np.int16)
    dstl_a[ks // (NT * 2), (ks // 2) % NT, ks % 2, j_within] = dstl[order]

    CPT = 2 * CPH
    TSLOT = CPT * 128
    idxA, idxB, dstlT, dstlF = [], [], [], []
    for k in range(NC_N):
        ia = gidx[k, :, 0, :].reshape(-1)
        ib = gidx[k, :, 1, :].reshape(-1)
        wrapA = np.tile(ia.reshape(-1, 16).T, (8, 1))
        wrapB = np.tile(ib.reshape(-1, 16).T, (8, 1))
        idxA.append(np.ascontiguousarray(wrapA))
        idxB.append(np.ascontiguousarray(wrapB))
        dT2 = dstl_a[k].reshape(NT, 2, CPH, 128)     # [t, h, c_h, p]
        dT2 = dT2.reshape(NT, CPT, 128)              # [t, c, p]
        dstlT.append(np.ascontiguousarray(
            dT2.transpose(2, 0, 1).reshape(128, NT * CPT).astype(BF16)))
        dstlF.append(np.ascontiguousarray(dT2.reshape(NT, TSLOT).astype(BF16)))

    return {
        "CPH": CPH,
        "slot_g": slot_g,
        "idxA": idxA, "idxB": idxB,
        "dstlT": dstlT, "dstlF": dstlF,
    }


def _fold_w(W, al, ar):
    """Wc = [Wl | W | Wr]: el = h@Wl, ft = h@W, er = h@Wr."""
    Din = W.shape[0]
    H, C = al.shape
    W3 = W.reshape(Din, H, C)
    Wl = np.einsum("dhc,hc->dh", W3, al)
    Wr = np.einsum("dhc,hc->dh", W3, ar)
    return np.concatenate([Wl, W, Wr], axis=1).astype(BF16)  # [Din, H + H*C + H]


# ----------------------------------------------------------------------------
# Device program
# ----------------------------------------------------------------------------

def _build_program(CPH, stages=None):
    import concourse.bass as bass
    import concourse.mybir as mybir
    import concourse.tile as tile
    from concourse import bacc
    from concourse.masks import make_identity

    f32 = mybir.dt.float32
    bf16 = mybir.dt.bfloat16
    fp8 = mybir.dt.float8e4
    i16 = mybir.dt.int16
    Alu = mybir.AluOpType
    Act = mybir.ActivationFunctionType

    CPT = 2 * CPH
    TSLOT = CPT * 128
    ICOL = NT * CPH * 8          # idx cols per half: NT*CPH*128/16

    nc = bacc.Bacc("TRN2", target_bir_lowering=False, debug=False,
                   num_devices=NC_N, num_swdge_queues=4)

    # ---- I/O ----
    xT = nc.dram_tensor("xT", [128, SLOTS], bf16, kind="ExternalInput")
    wc0 = nc.dram_tensor("wc0", [128, 272], bf16, kind="ExternalInput")
    wc1 = nc.dram_tensor("wc1", [256, 272], bf16, kind="ExternalInput")
    wc2 = nc.dram_tensor("wc2", [256, 34], bf16, kind="ExternalInput")
    idxA = nc.dram_tensor("idxA", [128, ICOL], i16, kind="ExternalInput")
    idxB = nc.dram_tensor("idxB", [128, ICOL], i16, kind="ExternalInput")
    dstlT = nc.dram_tensor("dstlT", [128, NT * CPT], bf16, kind="ExternalInput")
    dstlF = nc.dram_tensor("dstlF", [NT, TSLOT], bf16, kind="ExternalInput")
    iota128 = nc.dram_tensor("iota128", [128, 128], bf16, kind="ExternalInput")
    iotaPb = nc.dram_tensor("iotaPb", [128, 1], bf16, kind="ExternalInput")
    ones128 = nc.dram_tensor("ones128", [128, 128], bf16, kind="ExternalInput")
    out_d = nc.dram_tensor("out", [SLOTS, 32], f32, kind="ExternalOutput")

    # ---- internal DRAM ----
    ohC = nc.dram_tensor("ohC", [128, NT, 2 * TSLOT], fp8)
    T0s = nc.dram_tensor("T0s", [SLOTS, 512], fp8)
    T1s = nc.dram_tensor("T1s", [SLOTS, 512], fp8)
    T2s = nc.dram_tensor("T2s", [SLOTS, 128], bf16)
    T0f = nc.dram_tensor("T0f", [VTOT, 512], fp8, addr_space="Shared")
    T1f = nc.dram_tensor("T1f", [VTOT, 512], fp8, addr_space="Shared")
    T2f = nc.dram_tensor("T2f", [VTOT, 128], bf16, addr_space="Shared")

    def bc(ap, dims):
        return bass.AP(ap.tensor, ap.offset, [ap.ap[0]] + dims)

    with tile.TileContext(nc) as tc:
        import contextlib
        ctx = contextlib.ExitStack()
        with ctx:
            consts = ctx.enter_context(tc.tile_pool(name="consts", bufs=1))
            persist = ctx.enter_context(tc.tile_pool(name="persist", bufs=1))
            gpool = ctx.enter_context(tc.tile_pool(name="gather", bufs=3))
            mpool = ctx.enter_context(tc.tile_pool(name="msg", bufs=2))
            opool = ctx.enter_context(tc.tile_pool(name="oneh", bufs=3))
            spool = ctx.enter_context(tc.tile_pool(name="small", bufs=3))
            tpool = ctx.enter_context(tc.tile_pool(name="tbuild", bufs=2))
            pp_tb = ctx.enter_context(tc.tile_pool(name="ps_tb", bufs=2, space="PSUM"))
            pp_erd = ctx.enter_context(tc.tile_pool(name="ps_erd", bufs=2, space="PSUM"))
            pp_agg = ctx.enter_context(tc.tile_pool(name="ps_agg", bufs=2, space="PSUM"))
            pp_rep = ctx.enter_context(tc.tile_pool(name="ps_rep", bufs=1, space="PSUM"))
            pp_tp = ctx.enter_context(tc.tile_pool(name="ps_tp", bufs=1, space="PSUM"))

            # ---- load constants ----
            iota128_sb = consts.tile([128, 128], bf16)
            nc.sync.dma_start(out=iota128_sb[:], in_=iota128[:, :])
            iotaPb_sb = consts.tile([128, 1], bf16)
            nc.sync.dma_start(out=iotaPb_sb[:], in_=iotaPb[:, :])
            ones128_sb = consts.tile([128, 128], bf16)
            nc.sync.dma_start(out=ones128_sb[:], in_=ones128[:, :])
            ident = consts.tile([128, 128], bf16)
            make_identity(nc, ident[:])
            zer_bf = consts.tile([128, 1], bf16)
            nc.gpsimd.memset(zer_bf[:], 0)
            zer_f32 = consts.tile([128, 1], f32)
            nc.gpsimd.memset(zer_f32[:], 0)
            eps_f32 = consts.tile([128, 1], f32)
            nc.gpsimd.memset(eps_f32[:], 1e-30)

            idxA_sb = persist.tile([128, ICOL], i16)
            nc.sync.dma_start(out=idxA_sb[:], in_=idxA[:, :])
            idxB_sb = persist.tile([128, ICOL], i16)
            nc.sync.dma_start(out=idxB_sb[:], in_=idxB[:, :])
            dstlT_sb = persist.tile([128, NT * CPT], bf16)
            nc.sync.dma_start(out=dstlT_sb[:], in_=dstlT[:, :])
            xT_sb = persist.tile([128, SLOTS], bf16)
            nc.sync.dma_start(out=xT_sb[:], in_=xT[:, :])
            wc0_sb = persist.tile([128, 272], bf16)
            nc.sync.dma_start(out=wc0_sb[:], in_=wc0[:, :])
            wc1_sb = persist.tile([128, 2, 272], bf16)
            nc.sync.dma_start(out=wc1_sb[:, 0, :], in_=wc1[0:128, :])
            nc.sync.dma_start(out=wc1_sb[:, 1, :], in_=wc1[128:256, :])
            wc2_sb = persist.tile([128, 2, 34], bf16)
            nc.sync.dma_start(out=wc2_sb[:, 0, :], in_=wc2[0:128, :])
            nc.sync.dma_start(out=wc2_sb[:, 1, :], in_=wc2[128:256, :])

            # per-layer dst-side factors: [B=exp(er) | D=exp(.2 er)]
            er0_sb = persist.tile([128, NT, 16], bf16, tag="er0")
            er1_sb = persist.tile([128, NT, 16], bf16, tag="er1")
            er2_sb = persist.tile([128, NT, 2], bf16, tag="er2")
            hT_sb = persist.tile([128, 2, SLOTS], bf16, tag="hT")

            qn = [0]

            def next_q():
                qn[0] = (qn[0] + 1) % 4
                return qn[0]

            gsem = [nc.alloc_semaphore(f"gsem{q}") for q in range(4)]

            # ---------------- one-hot build (once, fp8) ----------------
            def build_onehots():
                for t in range(NT):
                    oh = opool.tile([128, CPT, 128], fp8, tag="ohi")
                    dT = dstlT_sb[:, t * CPT:(t + 1) * CPT]
                    nc.vector.tensor_tensor(
                        out=oh[:],
                        in0=bc(dT, [[dT.ap[1][0], CPT], [0, 128]]),
                        in1=bc(iota128_sb[:], [[0, CPT], [1, 128]]),
                        op=Alu.is_equal)
                    nc.sync.dma_start(
                        out=ohC[:, t, 0:TSLOT],
                        in_=oh[:].rearrange("p c e -> p (c e)"))
                    QS = 512
                    ohT = opool.tile([128, CPT, 128], fp8, tag="ohTi")
                    stage = spool.tile([1, TSLOT], bf16, tag="stage")
                    nc.sync.dma_start(out=stage[:], in_=dstlF[t:t + 1, :])
                    ohT_flat = ohT[:].rearrange("p c e -> p (c e)")
                    for s0 in range(0, TSLOT, QS):
                        s1 = min(s0 + QS, TSLOT)
                        rep = pp_rep.tile([128, QS], f32, tag="rep", space="PSUM")
                        nc.tensor.matmul(out=rep[:, 0:s1 - s0],
                                         lhsT=ones128_sb[0:1, :],
                                         rhs=stage[:, s0:s1],
                                         start=True, stop=True)
                        nc.vector.tensor_tensor(
                            out=ohT_flat[:, s0:s1],
                            in0=bc(iotaPb_sb[:, 0:1], [[0, s1 - s0]]),
                            in1=rep[:, 0:s1 - s0],
                            op=Alu.is_equal)
                    nc.sync.dma_start(
                        out=ohC[:, t, TSLOT:2 * TSLOT],
                        in_=ohT_flat)

            # ---------------- table build ----------------
            # ps cols: [el(H) | ft(FT) | er(H)]
            # L0/L1 row (fp8): [A(H) bf16 | C(H) bf16 | ft(FT) fp8]
            # L2 row (bf16):   [A(1) | C(1) | ft(32)]
            def build_table_tile(layer, t, Ts, er_out, H, FT):
                ps = pp_tb.tile([128, 2 * H + FT], f32, tag="tb_ps", space="PSUM")
                if layer == 0:
                    nc.tensor.matmul(out=ps[:], lhsT=xT_sb[:, t * 128:(t + 1) * 128],
                                     rhs=wc0_sb[:], start=True, stop=True)
                else:
                    w = wc1_sb if layer == 1 else wc2_sb
                    for kb in range(2):
                        nc.tensor.matmul(out=ps[:],
                                         lhsT=hT_sb[:, kb, t * 128:(t + 1) * 128],
                                         rhs=w[:, kb, :],
                                         start=(kb == 0), stop=(kb == 1))
                if FT == 256:
                    tsb = tpool.tile([128, 288], fp8, tag="tsb")
                    acA = tsb[:, 0:2 * H].bitcast(bf16)
                    acC = tsb[:, 2 * H:4 * H].bitcast(bf16)
                    ftv = tsb[:, 4 * H:4 * H + FT]
                    nwr = 288
                else:
                    tsb = tpool.tile([128, 34], bf16, tag="tsb2")
                    acA = tsb[:, 0:H]
                    acC = tsb[:, H:2 * H]
                    ftv = tsb[:, 2 * H:2 * H + FT]
                    nwr = 34
                nc.scalar.activation(out=acA, in_=ps[:, 0:H], func=Act.Exp)
                nc.scalar.activation(out=acC, in_=ps[:, 0:H],
                                     func=Act.Exp, scale=NEG_SLOPE)
                nc.scalar.activation(out=ftv, in_=ps[:, H:H + FT], func=Act.Copy)
                nc.scalar.activation(out=er_out[:, t, 0:H], in_=ps[:, H + FT:2 * H + FT],
                                     func=Act.Exp)
                nc.scalar.activation(out=er_out[:, t, H:2 * H], in_=ps[:, H + FT:2 * H + FT],
                                     func=Act.Exp, scale=NEG_SLOPE)
                nc.sync.dma_start(out=Ts[t * 128:(t + 1) * 128, 0:nwr],
                                  in_=tsb[:, 0:nwr])

            def allgather(Ts, Tf, ncol):
                nc.gpsimd.collective_compute(
                    "AllGather", Alu.bypass,
                    replica_groups=[list(range(NC_N))],
                    ins=[Ts[:, :]], outs=[Tf[:, :]],
                )

            # ---------------- edge phase ----------------
            def edge_phase(layer, Tf, er_sb, H, FT):
                ROW = 512 if FT == 256 else 128     # fp8 cols / bf16 cols
                gdt = fp8 if FT == 256 else bf16
                MSGN = FT + H
                ftc0 = 4 * H if FT == 256 else 2 * H
                for t in range(NT):
                    # gathers (A/B halves) into one [128, CPT, ROW] tile,
                    # prepared ahead on Q7 and fired when Tf is ready
                    # Split each half-gather into 3-chunk pieces so one call's
                    # descriptors (384) fit the 1024-desc ring carveout: Q7
                    # emission never stalls on its own drain, and all 4 queues
                    # drain concurrently.
                    g = gpool.tile([128, CPT, ROW], gdt, tag="g")
                    cols = CPH * 8
                    GS = CPH                        # chunks per gather call
                    for half, base in ((0, 0), (1, HALF)):
                        for c0 in range(0, CPH, GS):
                            c1 = min(c0 + GS, CPH)
                            idx = idxA_sb if half == 0 else idxB_sb
                            nc.gpsimd.dma_gather(
                                out_ap=g[:, half * CPH + c0:half * CPH + c1, :],
                                in_ap=Tf[base:base + HALF, :],
                                idxs_ap=idx[:, t * cols + c0 * 8:t * cols + c1 * 8],
                                num_idxs=(c1 - c0) * 128,
                                num_idxs_reg=(c1 - c0) * 128,
                                elem_size=ROW, single_packet=False,
                                queue_num=next_q())

                    # stream fp8 one-hots (oh | ohT in one transfer)
                    ohx = opool.tile([128, 2, CPT, 128], fp8, tag="ohx")
                    nc.sync.dma_start(
                        out=ohx[:].rearrange("p a c e -> p (a c e)"),
                        in_=ohC[:, t, :])
                    oh = ohx[:, 0]
                    ohT = ohx[:, 1]

                    # expand dst factors per edge: erd[e, c, 0:H]=B, [H:2H]=D
                    erd = pp_erd.tile([128, CPT, 2 * H], f32, tag="erd",
                                      space="PSUM")
                    for c in range(CPT):
                        nc.tensor.matmul(out=erd[:, c, :],
                                         lhsT=ohT[:, c, :],
                                         rhs=er_sb[:, t, 0:2 * H],
                                         start=True, stop=True)

                    # p = max(A*B, C*D)   [128, CPT, H] bf16
                    if FT == 256:
                        gA = g[:, :, 0:2 * H].bitcast(bf16)
                        gC = g[:, :, 2 * H:4 * H].bitcast(bf16)
                    else:
                        gA = g[:, :, 0:H]
                        gC = g[:, :, H:2 * H]
                    m1 = spool.tile([128, CPT, H], bf16, tag="m1")
                    m2 = spool.tile([128, CPT, H], bf16, tag="m2")
                    nc.vector.tensor_tensor(
                        out=m1[:], in0=gA,
                        in1=erd[:, :, 0:H], op=Alu.mult)
                    nc.vector.tensor_tensor(
                        out=m2[:], in0=gC,
                        in1=erd[:, :, H:2 * H], op=Alu.mult)
                    p = spool.tile([128, CPT, H], bf16, tag="p")
                    nc.vector.tensor_tensor(out=p[:], in0=m1[:], in1=m2[:],
                                            op=Alu.max)

                    # messages: p*ft (denominator p accumulated separately)
                    msg = mpool.tile([128, CPT, FT], bf16, tag="msg")
                    CW = FT // H                    # channels per head
                    nc.vector.tensor_tensor(
                        out=bc(msg[:], [[FT, CPT], [CW, H], [1, CW]]),
                        in0=bc(g[:, :, ftc0:ftc0 + FT],
                               [[ROW, CPT], [CW, H], [1, CW]]),
                        in1=bc(p[:], [[H, CPT], [1, H], [0, CW]]),
                        op=Alu.mult)

                    # segment reduction: [sum p*ft | sum p]
                    agg = pp_agg.tile([128, MSGN], f32, tag="agg", space="PSUM")
                    for c in range(CPT):
                        nc.tensor.matmul(out=agg[:, 0:FT], lhsT=oh[:, c, :],
                                         rhs=msg[:, c, :],
                                         start=(c == 0), stop=(c == CPT - 1))
                    for c in range(CPT):
                        nc.tensor.matmul(out=agg[:, FT:FT + H], lhsT=oh[:, c, :],
                                         rhs=p[:, c, :],
                                         start=(c == 0), stop=(c == CPT - 1))

                    # epilogue
                    s_sb = spool.tile([128, H], f32, tag="s")
                    nc.vector.tensor_tensor(out=s_sb[:], in0=agg[:, FT:FT + H],
                                            in1=bc(eps_f32[:, 0:1], [[0, H]]),
                                            op=Alu.add)
                    rs = spool.tile([128, H], f32, tag="rs")
                    nc.vector.reciprocal(out=rs[:], in_=s_sb[:])
                    if layer < 2:
                        h_t = spool.tile([128, 256], bf16, tag="ht")
                        nc.vector.tensor_tensor(
                            out=bc(h_t[:], [[32, H], [1, 32]]),
                            in0=bc(agg[:, 0:FT], [[32, H], [1, 32]]),
                            in1=bc(rs[:], [[1, H], [0, 32]]),
                            op=Alu.mult)
                        nc.vector.tensor_tensor(out=h_t[:], in0=h_t[:],
                                                in1=bc(zer_bf[:, 0:1], [[0, 256]]),
                                                op=Alu.max)
                        for b in range(2):
                            tp = pp_tp.tile([128, 128], bf16, tag="tp",
                                            space="PSUM")
                            nc.tensor.transpose(out=tp[:],
                                                in_=h_t[:, b * 128:(b + 1) * 128],
                                                identity=ident[:])
                            nc.scalar.activation(
                                out=hT_sb[:, b, t * 128:(t + 1) * 128],
                                in_=tp[:], func=Act.Copy)
                        # fused next-layer table build for this tile
                        if layer == 0:
                            build_table_tile(1, t, T1s, er1_sb, 8, 256)
                        else:
                            build_table_tile(2, t, T2s, er2_sb, 1, 32)
                    else:
                        o1 = spool.tile([128, 32], f32, tag="o1")
                        nc.vector.tensor_tensor(
                            out=o1[:], in0=agg[:, 0:32],
                            in1=bc(rs[:, 0:1], [[0, 32]]), op=Alu.mult)
                        nc.vector.tensor_tensor(out=o1[:], in0=o1[:],
                                                in1=bc(zer_f32[:, 0:1], [[0, 32]]),
                                                op=Alu.max)
                        nc.scalar.activation(out=o1[:], in_=o1[:], func=Act.Exp)
                        ssum = spool.tile([128, 1], f32, tag="ssum")
                        nc.vector.tensor_reduce(out=ssum[:], in_=o1[:],
                                                axis=mybir.AxisListType.X,
                                                op=Alu.add)
                        rr = spool.tile([128, 1], f32, tag="rr")
                        nc.vector.reciprocal(out=rr[:], in_=ssum[:])
                        ofin = spool.tile([128, 32], f32, tag="ofin")
                        nc.vector.tensor_tensor(out=ofin[:], in0=o1[:],
                                                in1=bc(rr[:], [[0, 32]]),
                                                op=Alu.mult)
                        nc.sync.dma_start(out=out_d[t * 128:(t + 1) * 128, :],
                                          in_=ofin[:])

            # ================= schedule =================
            all_stages = ["t0", "ag0", "oh", "e0", "ag1", "e1", "ag2", "e2"]
            st = all_stages if stages is None else stages
            if "t0" in st:
                for t in range(NT):
                    build_table_tile(0, t, T0s, er0_sb, 8, 256)
            if "ag0" in st:
                allgather(T0s, T0f, 272)
            if "oh" in st:
                build_onehots()
            if "e0" in st:
                edge_phase(0, T0f, er0_sb, 8, 256)
            if "ag1" in st:
                allgather(T1s, T1f, 272)
            if "e1" in st:
                edge_phase(1, T1f, er1_sb, 8, 256)
            if "ag2" in st:
                allgather(T2s, T2f, 34)
            if "e2" in st:
                edge_phase(2, T2f, er2_sb, 1, 32)

    nc.compile()
    return nc


# ----------------------------------------------------------------------------
# Entry point
# ----------------------------------------------------------------------------

def kernel(x, src, dst, W0, al0, ar0, b0, W1, al1, ar1, b1, W2, al2, ar2, b2):
    from concourse.bass_utils import run_bass_kernel_spmd

    x = np.asarray(x, dtype=np.float32)
    g = _prep_graph(src, dst)
    CPH = g["CPH"]

    key = ("prog", CPH)
    if key not in _CACHE:
        _CACHE[key] = _build_program(CPH)
    nc = _CACHE[key]

    wc0 = _fold_w(np.asarray(W0, np.float32), np.asarray(al0, np.float32),
                  np.asarray(ar0, np.float32))
    wc1 = _fold_w(np.asarray(W1, np.float32), np.asarray(al1, np.float32),
                  np.asarray(ar1, np.float32))
    wc2 = _fold_w(np.asarray(W2, np.float32), np.asarray(al2, np.float32),
                  np.asarray(ar2, np.float32))

    iota128 = np.broadcast_to(np.arange(128, dtype=np.float32), (128, 128)) \
        .astype(BF16)
    iotaPb = np.arange(128, dtype=np.float32).reshape(128, 1).astype(BF16)
    ones128 = np.ones((128, 128), dtype=BF16)

    slot_g = g["slot_g"]
    in_maps = []
    for k in range(NC_N):
        lo, hi = k * NPC, (k + 1) * NPC
        xTk = np.zeros((128, SLOTS), dtype=BF16)
        xTk[:, slot_g[lo:hi]] = x[lo:hi].T.astype(BF16)
        in_maps.append({
            "xT": np.ascontiguousarray(xTk),
            "wc0": wc0, "wc1": wc1, "wc2": wc2,
            "idxA": g["idxA"][k], "idxB": g["idxB"][k],
            "dstlT": g["dstlT"][k], "dstlF": g["dstlF"][k],
            "iota128": iota128, "iotaPb": iotaPb,
            "ones128": ones128,
        })

    global _last_in_maps
    _last_in_maps = in_maps
    res = run_bass_kernel_spmd(nc, in_maps, core_ids=list(range(NC_N)))

    out = np.empty((N_NODES, 32), dtype=np.float32)
    for k in range(NC_N):
        lo, hi = k * NPC, (k + 1) * NPC
        out[lo:hi] = res.results[k]["out"][slot_g[lo:hi]]
    return out


# revision 24
# speedup vs baseline: 1.3188x; 1.0660x over previous
"""3-layer GAT on 8 Trainium2 NeuronCores — v2.

Strategy (edge-parallel by destination), changes vs v1:
- exp(lrelu(el+er)) = max(exp(el)exp(er), exp(.2el)exp(.2er)): the table
  stores A=exp(el), C=exp(.2el) bf16 (no f32 logits, no Exp in edge phase);
  per-dst B=exp(er), D=exp(.2er) live in SBUF and are expanded per edge by
  the ohT matmul.  p = max(A*B, C*D) on DVE.
- One-hot matrices are fp8e4 (0/1 exact; matmul allows fp8 lhsT with bf16
  rhs), generated once on device, staged in DRAM, streamed per tile-layer:
  halves one-hot HBM traffic ~4x vs v1 bf16 oh+ohT.
- PSUM->SBUF casts run on the Scalar (ACT) engine, freeing DVE.
- Next-layer table build is fused into the edge-phase tile loop, so the
  AllGather launches immediately when the last tile finishes.
"""

import numpy as np
import ml_dtypes

N_NODES = 50000
N_EDGES = 800000
IN_FEATS = 128
HID = 32
HEADS = 8
OUT_FEATS = 32
NEG_SLOPE = 0.2

NC_N = 8                 # cores
NPC = N_NODES // NC_N    # real nodes per core (6250)
NT = 49                  # dst tiles per core
SLOTS = NT * 128         # 6272 slots per core
HALF = 4 * SLOTS         # 25088 table rows per half
VTOT = NC_N * SLOTS      # 50176 table rows

BF16 = ml_dtypes.bfloat16

_CACHE = {}
_last_in_maps = None


# ----------------------------------------------------------------------------
# Host-side graph preparation (same as v1)
# ----------------------------------------------------------------------------

def _prep_graph(src, dst):
    src = np.asarray(src).astype(np.int64)
    dst = np.asarray(dst).astype(np.int64)

    ecore = dst // NPC

    slot_g = np.zeros(N_NODES, dtype=np.int64)
    degA = np.zeros(N_NODES, dtype=np.int64)
    degB = np.zeros(N_NODES, dtype=np.int64)
    half_e = (src >= 4 * NPC).astype(np.int64)
    np.add.at(degA, dst[half_e == 0], 1)
    np.add.at(degB, dst[half_e == 1], 1)

    for k in range(NC_N):
        lo, hi = k * NPC, (k + 1) * NPC
        nodes = np.arange(lo, hi)
        d = degA[lo:hi] + degB[lo:hi]
        order = np.argsort(-d, kind="stable")
        loads = np.zeros(NT, dtype=np.int64)
        counts = np.zeros(NT, dtype=np.int64)
        tile_of = np.zeros(NPC, dtype=np.int64)
        pos_of = np.zeros(NPC, dtype=np.int64)
        for i in order:
            t = np.argmin(np.where(counts < 128, loads, np.iinfo(np.int64).max))
            tile_of[i] = t
            pos_of[i] = counts[t]
            counts[t] += 1
            loads[t] += d[i]
        slot_g[nodes] = tile_of * 128 + pos_of

    srcslot = (src // NPC) * SLOTS + slot_g[src]
    dslot = slot_g[dst]
    dtile = dslot // 128
    dstl = dslot % 128

    key = (ecore * NT + dtile) * 2 + half_e
    # sort by src row within each group: ascending gather addresses
    order = np.lexsort((srcslot, key))
    key_s = key[order]
    ngroups = NC_N * NT * 2
    counts = np.bincount(key_s, minlength=ngroups)
    starts = np.concatenate([[0], np.cumsum(counts)[:-1]])
    j_within = np.arange(len(src)) - starts[key_s]

    CPH = int(np.ceil(counts.max() / 128))
    CAP = CPH * 128

    gidx = np.zeros((NC_N, NT, 2, CAP), dtype=np.int16)
    dstl_a = np.full((NC_N, NT, 2, CAP), -1.0, dtype=np.float32)

    ks = key_s
    gidx[ks // (NT * 2), (ks // 2) % NT, ks % 2, j_within] = (
        srcslot[order] - (ks % 2) * HALF
    ).astype(np.int16)
    dstl_a[ks // (NT * 2), (ks // 2) % NT, ks % 2, j_within] = dstl[order]

    CPT = 2 * CPH
    TSLOT = CPT * 128
    idxA, idxB, dstlT, dstlF = [], [], [], []
    for k in range(NC_N):
        ia = gidx[k, :, 0, :].reshape(-1)
        ib = gidx[k, :, 1, :].reshape(-1)
        wrapA = np.tile(ia.reshape(-1, 16).T, (8, 1))
        wrapB = np.tile(ib.reshape(-1, 16).T, (8, 1))
        idxA.append(np.ascontiguousarray(wrapA))
        idxB.append(np.ascontiguousarray(wrapB))
        dT2 = dstl_a[k].reshape(NT, 2, CPH, 128)     # [t, h, c_h, p]
        dT2 = dT2.reshape(NT, CPT, 128)              # [t, c, p]
        dstlT.append(np.ascontiguousarray(
            dT2.transpose(2, 0, 1).reshape(128, NT * CPT).astype(BF16)))
        dstlF.append(np.ascontiguousarray(dT2.reshape(NT, TSLOT).astype(BF16)))

    return {
        "CPH": CPH,
        "slot_g": slot_g,
        "idxA": idxA, "idxB": idxB,
        "dstlT": dstlT, "dstlF": dstlF,
    }


def _fold_w(W, al, ar):
    """Wc = [Wl | W | Wr]: el = h@Wl, ft = h@W, er = h@Wr."""
    Din = W.shape[0]
    H, C = al.shape
    W3 = W.reshape(Din, H, C)
    Wl = np.einsum("dhc,hc->dh", W3, al)
    Wr = np.einsum("dhc,hc->dh", W3, ar)
    return np.concatenate([Wl, W, Wr], axis=1).astype(BF16)  # [Din, H + H*C + H]


# ----------------------------------------------------------------------------
# Device program
# ----------------------------------------------------------------------------

def _build_program(CPH, stages=None):
    import concourse.bass as bass
    import concourse.mybir as mybir
    import concourse.tile as tile
    from concourse import bacc
    from concourse.masks import make_identity

    f32 = mybir.dt.float32
    bf16 = mybir.dt.bfloat16
    fp8 = mybir.dt.float8e4
    i16 = mybir.dt.int16
    Alu = mybir.AluOpType
    Act = mybir.ActivationFunctionType

    CPT = 2 * CPH
    TSLOT = CPT * 128
    ICOL = NT * CPH * 8          # idx cols per half: NT*CPH*128/16

    nc = bacc.Bacc("TRN2", target_bir_lowering=False, debug=False,
                   num_devices=NC_N, num_swdge_queues=4)

    # ---- I/O ----
    xT = nc.dram_tensor("xT", [128, SLOTS], bf16, kind="ExternalInput")
    wc0 = nc.dram_tensor("wc0", [128, 272], bf16, kind="ExternalInput")
    wc1 = nc.dram_tensor("wc1", [256, 272], bf16, kind="ExternalInput")
    wc2 = nc.dram_tensor("wc2", [256, 34], bf16, kind="ExternalInput")
    idxA = nc.dram_tensor("idxA", [128, ICOL], i16, kind="ExternalInput")
    idxB = nc.dram_tensor("idxB", [128, ICOL], i16, kind="ExternalInput")
    dstlT = nc.dram_tensor("dstlT", [128, NT * CPT], bf16, kind="ExternalInput")
    dstlF = nc.dram_tensor("dstlF", [NT, TSLOT], bf16, kind="ExternalInput")
    iota128 = nc.dram_tensor("iota128", [128, 128], bf16, kind="ExternalInput")
    iotaPb = nc.dram_tensor("iotaPb", [128, 1], bf16, kind="ExternalInput")
    ones128 = nc.dram_tensor("ones128", [128, 128], bf16, kind="ExternalInput")
    out_d = nc.dram_tensor("out", [SLOTS, 32], f32, kind="ExternalOutput")

    # ---- internal DRAM ----
    ohC = nc.dram_tensor("ohC", [128, NT, 2 * TSLOT], fp8)
    T0s = nc.dram_tensor("T0s", [SLOTS, 512], fp8)
    T1s = nc.dram_tensor("T1s", [SLOTS, 512], fp8)
    T2s = nc.dram_tensor("T2s", [SLOTS, 128], bf16)
    T0f = nc.dram_tensor("T0f", [VTOT, 512], fp8, addr_space="Shared")
    T1f = nc.dram_tensor("T1f", [VTOT, 512], fp8, addr_space="Shared")
    T2f = nc.dram_tensor("T2f", [VTOT, 128], bf16, addr_space="Shared")

    def bc(ap, dims):
        return bass.AP(ap.tensor, ap.offset, [ap.ap[0]] + dims)

    with tile.TileContext(nc) as tc:
        import contextlib
        ctx = contextlib.ExitStack()
        with ctx:
            consts = ctx.enter_context(tc.tile_pool(name="consts", bufs=1))
            persist = ctx.enter_context(tc.tile_pool(name="persist", bufs=1))
            gpool = ctx.enter_context(tc.tile_pool(name="gather", bufs=5))
            mpool = ctx.enter_context(tc.tile_pool(name="msg", bufs=3))
            opool = ctx.enter_context(tc.tile_pool(name="oneh", bufs=4))
            spool = ctx.enter_context(tc.tile_pool(name="small", bufs=4))
            tpool = ctx.enter_context(tc.tile_pool(name="tbuild", bufs=2))
            pp_tb = ctx.enter_context(tc.tile_pool(name="ps_tb", bufs=2, space="PSUM"))
            pp_erd = ctx.enter_context(tc.tile_pool(name="ps_erd", bufs=2, space="PSUM"))
            pp_agg = ctx.enter_context(tc.tile_pool(name="ps_agg", bufs=2, space="PSUM"))
            pp_rep = ctx.enter_context(tc.tile_pool(name="ps_rep", bufs=1, space="PSUM"))
            pp_tp = ctx.enter_context(tc.tile_pool(name="ps_tp", bufs=1, space="PSUM"))

            # ---- load constants ----
            iota128_sb = consts.tile([128, 128], bf16)
            nc.sync.dma_start(out=iota128_sb[:], in_=iota128[:, :])
            iotaPb_sb = consts.tile([128, 1], bf16)
            nc.sync.dma_start(out=iotaPb_sb[:], in_=iotaPb[:, :])
            ones128_sb = consts.tile([128, 128], bf16)
            nc.sync.dma_start(out=ones128_sb[:], in_=ones128[:, :])
            ident = consts.tile([128, 128], bf16)
            make_identity(nc, ident[:])
            zer_bf = consts.tile([128, 1], bf16)
            nc.gpsimd.memset(zer_bf[:], 0)
            zer_f32 = consts.tile([128, 1], f32)
            nc.gpsimd.memset(zer_f32[:], 0)
            eps_f32 = consts.tile([128, 1], f32)
            nc.gpsimd.memset(eps_f32[:], 1e-30)

            idxA_sb = persist.tile([128, ICOL], i16)
            nc.sync.dma_start(out=idxA_sb[:], in_=idxA[:, :])
            idxB_sb = persist.tile([128, ICOL], i16)
            nc.sync.dma_start(out=idxB_sb[:], in_=idxB[:, :])
            dstlT_sb = persist.tile([128, NT * CPT], bf16)
            nc.sync.dma_start(out=dstlT_sb[:], in_=dstlT[:, :])
            xT_sb = persist.tile([128, SLOTS], bf16)
            nc.sync.dma_start(out=xT_sb[:], in_=xT[:, :])
            wc0_sb = persist.tile([128, 272], bf16)
            nc.sync.dma_start(out=wc0_sb[:], in_=wc0[:, :])
            wc1_sb = persist.tile([128, 2, 272], bf16)
            nc.sync.dma_start(out=wc1_sb[:, 0, :], in_=wc1[0:128, :])
            nc.sync.dma_start(out=wc1_sb[:, 1, :], in_=wc1[128:256, :])
            wc2_sb = persist.tile([128, 2, 34], bf16)
            nc.sync.dma_start(out=wc2_sb[:, 0, :], in_=wc2[0:128, :])
            nc.sync.dma_start(out=wc2_sb[:, 1, :], in_=wc2[128:256, :])

            # per-layer dst-side factors: [B=exp(er) | D=exp(.2 er)]
            er0_sb = persist.tile([128, NT, 16], bf16, tag="er0")
            er1_sb = persist.tile([128, NT, 16], bf16, tag="er1")
            er2_sb = persist.tile([128, NT, 2], bf16, tag="er2")
            hT_sb = persist.tile([128, 2, SLOTS], bf16, tag="hT")

            qn = [0]

            def next_q():
                qn[0] = (qn[0] + 1) % 4
                return qn[0]

            gsem = [nc.alloc_semaphore(f"gsem{q}") for q in range(4)]

            # ---------------- one-hot build (once, fp8) ----------------
            def build_onehots():
                for t in range(NT):
                    oh = opool.tile([128, CPT, 128], fp8, tag="ohi")
                    dT = dstlT_sb[:, t * CPT:(t + 1) * CPT]
                    nc.vector.tensor_tensor(
                        out=oh[:],
                        in0=bc(dT, [[dT.ap[1][0], CPT], [0, 128]]),
                        in1=bc(iota128_sb[:], [[0, CPT], [1, 128]]),
                        op=Alu.is_equal)
                    nc.sync.dma_start(
                        out=ohC[:, t, 0:TSLOT],
                        in_=oh[:].rearrange("p c e -> p (c e)"))
                    QS = 512
                    ohT = opool.tile([128, CPT, 128], fp8, tag="ohTi")
                    stage = spool.tile([1, TSLOT], bf16, tag="stage")
                    nc.sync.dma_start(out=stage[:], in_=dstlF[t:t + 1, :])
                    ohT_flat = ohT[:].rearrange("p c e -> p (c e)")
                    for s0 in range(0, TSLOT, QS):
                        s1 = min(s0 + QS, TSLOT)
                        rep = pp_rep.tile([128, QS], f32, tag="rep", space="PSUM")
                        nc.tensor.matmul(out=rep[:, 0:s1 - s0],
                                         lhsT=ones128_sb[0:1, :],
                                         rhs=stage[:, s0:s1],
                                         start=True, stop=True)
                        nc.vector.tensor_tensor(
                            out=ohT_flat[:, s0:s1],
                            in0=bc(iotaPb_sb[:, 0:1], [[0, s1 - s0]]),
                            in1=rep[:, 0:s1 - s0],
                            op=Alu.is_equal)
                    nc.sync.dma_start(
                        out=ohC[:, t, TSLOT:2 * TSLOT],
                        in_=ohT_flat)

            # ---------------- table build ----------------
            # ps cols: [el(H) | ft(FT) | er(H)]
            # L0/L1 row (fp8): [A(H) bf16 | C(H) bf16 | ft(FT) fp8]
            # L2 row (bf16):   [A(1) | C(1) | ft(32)]
            def build_table_tile(layer, t, Ts, er_out, H, FT):
                ps = pp_tb.tile([128, 2 * H + FT], f32, tag="tb_ps", space="PSUM")
                if layer == 0:
                    nc.tensor.matmul(out=ps[:], lhsT=xT_sb[:, t * 128:(t + 1) * 128],
                                     rhs=wc0_sb[:], start=True, stop=True)
                else:
                    w = wc1_sb if layer == 1 else wc2_sb
                    for kb in range(2):
                        nc.tensor.matmul(out=ps[:],
                                         lhsT=hT_sb[:, kb, t * 128:(t + 1) * 128],
                                         rhs=w[:, kb, :],
                                         start=(kb == 0), stop=(kb == 1))
                if FT == 256:
                    tsb = tpool.tile([128, 288], fp8, tag="tsb")
                    acA = tsb[:, 0:2 * H].bitcast(bf16)
                    acC = tsb[:, 2 * H:4 * H].bitcast(bf16)
                    ftv = tsb[:, 4 * H:4 * H + FT]
                    nwr = 288
                else:
                    tsb = tpool.tile([128, 34], bf16, tag="tsb2")
                    acA = tsb[:, 0:H]
                    acC = tsb[:, H:2 * H]
                    ftv = tsb[:, 2 * H:2 * H + FT]
                    nwr = 34
                nc.scalar.activation(out=acA, in_=ps[:, 0:H], func=Act.Exp)
                nc.scalar.activation(out=acC, in_=ps[:, 0:H],
                                     func=Act.Exp, scale=NEG_SLOPE)
                nc.scalar.activation(out=ftv, in_=ps[:, H:H + FT], func=Act.Copy)
                nc.scalar.activation(out=er_out[:, t, 0:H], in_=ps[:, H + FT:2 * H + FT],
                                     func=Act.Exp)
                nc.scalar.activation(out=er_out[:, t, H:2 * H], in_=ps[:, H + FT:2 * H + FT],
                                     func=Act.Exp, scale=NEG_SLOPE)
                nc.sync.dma_start(out=Ts[t * 128:(t + 1) * 128, 0:nwr],
                                  in_=tsb[:, 0:nwr])

            def allgather(Ts, Tf, ncol):
                nc.gpsimd.collective_compute(
                    "AllGather", Alu.bypass,
                    replica_groups=[list(range(NC_N))],
                    ins=[Ts[:, :]], outs=[Tf[:, :]],
                )

            # ---------------- edge phase ----------------
            def edge_phase(layer, Tf, er_sb, H, FT):
                ROW = 512 if FT == 256 else 128     # fp8 cols / bf16 cols
                gdt = fp8 if FT == 256 else bf16
                MSGN = FT + H
                ftc0 = 4 * H if FT == 256 else 2 * H
                for t in range(NT):
                    # gathers (A/B halves) into one [128, CPT, ROW] tile,
                    # prepared ahead on Q7 and fired when Tf is ready
                    # Split each half-gather into 3-chunk pieces so one call's
                    # descriptors (384) fit the 1024-desc ring carveout: Q7
                    # emission never stalls on its own drain, and all 4 queues
                    # drain concurrently.
                    g = gpool.tile([128, CPT, ROW], gdt, tag="g")
                    cols = CPH * 8
                    GS = 5                          # chunks per gather call
                    for half, base in ((0, 0), (1, HALF)):
                        for c0 in range(0, CPH, GS):
                            c1 = min(c0 + GS, CPH)
                            idx = idxA_sb if half == 0 else idxB_sb
                            nc.gpsimd.dma_gather(
                                out_ap=g[:, half * CPH + c0:half * CPH + c1, :],
                                in_ap=Tf[base:base + HALF, :],
                                idxs_ap=idx[:, t * cols + c0 * 8:t * cols + c1 * 8],
                                num_idxs=(c1 - c0) * 128,
                                num_idxs_reg=(c1 - c0) * 128,
                                elem_size=ROW, single_packet=False,
                                queue_num=next_q())

                    # stream fp8 one-hots (oh | ohT in one transfer)
                    ohx = opool.tile([128, 2, CPT, 128], fp8, tag="ohx")
                    nc.sync.dma_start(
                        out=ohx[:].rearrange("p a c e -> p (a c e)"),
                        in_=ohC[:, t, :])
                    oh = ohx[:, 0]
                    ohT = ohx[:, 1]

                    # expand dst factors per edge: erd[e, c, 0:H]=B, [H:2H]=D
                    erd = pp_erd.tile([128, CPT, 2 * H], f32, tag="erd",
                                      space="PSUM")
                    for c in range(CPT):
                        nc.tensor.matmul(out=erd[:, c, :],
                                         lhsT=ohT[:, c, :],
                                         rhs=er_sb[:, t, 0:2 * H],
                                         start=True, stop=True)

                    # p = max(A*B, C*D)   [128, CPT, H] bf16
                    if FT == 256:
                        gA = g[:, :, 0:2 * H].bitcast(bf16)
                        gC = g[:, :, 2 * H:4 * H].bitcast(bf16)
                    else:
                        gA = g[:, :, 0:H]
                        gC = g[:, :, H:2 * H]
                    m1 = spool.tile([128, CPT, H], bf16, tag="m1")
                    m2 = spool.tile([128, CPT, H], bf16, tag="m2")
                    nc.vector.tensor_tensor(
                        out=m1[:], in0=gA,
                        in1=erd[:, :, 0:H], op=Alu.mult)
                    nc.vector.tensor_tensor(
                        out=m2[:], in0=gC,
                        in1=erd[:, :, H:2 * H], op=Alu.mult)
                    p = spool.tile([128, CPT, H], bf16, tag="p")
                    nc.vector.tensor_tensor(out=p[:], in0=m1[:], in1=m2[:],
                                            op=Alu.max)

                    # messages: p*ft (denominator p accumulated separately)
                    msg = mpool.tile([128, CPT, FT], bf16, tag="msg")
                    CW = FT // H                    # channels per head
                    nc.vector.tensor_tensor(
                        out=bc(msg[:], [[FT, CPT], [CW, H], [1, CW]]),
                        in0=bc(g[:, :, ftc0:ftc0 + FT],
                               [[ROW, CPT], [CW, H], [1, CW]]),
                        in1=bc(p[:], [[H, CPT], [1, H], [0, CW]]),
                        op=Alu.mult)

                    # segment reduction: [sum p*ft | sum p]
                    agg = pp_agg.tile([128, MSGN], f32, tag="agg", space="PSUM")
                    for c in range(CPT):
                        nc.tensor.matmul(out=agg[:, 0:FT], lhsT=oh[:, c, :],
                                         rhs=msg[:, c, :],
                                         start=(c == 0), stop=(c == CPT - 1))
                    for c in range(CPT):
                        nc.tensor.matmul(out=agg[:, FT:FT + H], lhsT=oh[:, c, :],
                                         rhs=p[:, c, :],
                                         start=(c == 0), stop=(c == CPT - 1))

                    # epilogue
                    s_sb = spool.tile([128, H], f32, tag="s")
                    nc.vector.tensor_tensor(out=s_sb[:], in0=agg[:, FT:FT + H],
                                            in1=bc(eps_f32[:, 0:1], [[0, H]]),
                                            op=Alu.add)
                    rs = spool.tile([128, H], f32, tag="rs")
                    nc.vector.reciprocal(out=rs[:], in_=s_sb[:])
                    if layer < 2:
                        h_t = spool.tile([128, 256], bf16, tag="ht")
                        nc.vector.tensor_tensor(
                            out=bc(h_t[:], [[32, H], [1, 32]]),
                            in0=bc(agg[:, 0:FT], [[32, H], [1, 32]]),
                            in1=bc(rs[:], [[1, H], [0, 32]]),
                            op=Alu.mult)
                        nc.vector.tensor_tensor(out=h_t[:], in0=h_t[:],
                                                in1=bc(zer_bf[:, 0:1], [[0, 256]]),
                                                op=Alu.max)
                        for b in range(2):
                            tp = pp_tp.tile([128, 128], bf16, tag="tp",
                                            space="PSUM")
                            nc.tensor.transpose(out=tp[:],
                                                in_=h_t[:, b * 128:(b + 1) * 128],
                                                identity=ident[:])
                            nc.scalar.activation(
                                out=hT_sb[:, b, t * 128:(t + 1) * 128],
                                in_=tp[:], func=Act.Copy)
                        # fused next-layer table build for this tile
                        if layer == 0:
                            build_table_tile(1, t, T1s, er1_sb, 8, 256)
                        else:
                            build_table_tile(2, t, T2s, er2_sb, 1, 32)
                    else:
                        o1 = spool.tile([128, 32], f32, tag="o1")
                        nc.vector.tensor_tensor(
                            out=o1[:], in0=agg[:, 0:32],
                            in1=bc(rs[:, 0:1], [[0, 32]]), op=Alu.mult)
                        nc.vector.tensor_tensor(out=o1[:], in0=o1[:],
                                                in1=bc(zer_f32[:, 0:1], [[0, 32]]),
                                                op=Alu.max)
                        nc.scalar.activation(out=o1[:], in_=o1[:], func=Act.Exp)
                        ssum = spool.tile([128, 1], f32, tag="ssum")
                        nc.vector.tensor_reduce(out=ssum[:], in_=o1[:],
                                                axis=mybir.AxisListType.X,
                                                op=Alu.add)
                        rr = spool.tile([128, 1], f32, tag="rr")
                        nc.vector.reciprocal(out=rr[:], in_=ssum[:])
                        ofin = spool.tile([128, 32], f32, tag="ofin")
                        nc.vector.tensor_tensor(out=ofin[:], in0=o1[:],
                                                in1=bc(rr[:], [[0, 32]]),
                                                op=Alu.mult)
                        nc.sync.dma_start(out=out_d[t * 128:(t + 1) * 128, :],
                                          in_=ofin[:])

            # ================= schedule =================
            all_stages = ["t0", "ag0", "oh", "e0", "ag1", "e1", "ag2", "e2"]
            st = all_stages if stages is None else stages
            if "t0" in st:
                for t in range(NT):
                    build_table_tile(0, t, T0s, er0_sb, 8, 256)
            if "ag0" in st:
                allgather(T0s, T0f, 272)
            if "oh" in st:
                build_onehots()
            if "e0" in st:
                edge_phase(0, T0f, er0_sb, 8, 256)
            if "ag1" in st:
                allgather(T1s, T1f, 272)
            if "e1" in st:
                edge_phase(1, T1f, er1_sb, 8, 256)
            if "ag2" in st:
                allgather(T2s, T2f, 34)
            if "e2" in st:
                edge_phase(2, T2f, er2_sb, 1, 32)

    nc.compile()
    return nc


# ----------------------------------------------------------------------------
# Entry point
# ----------------------------------------------------------------------------

def kernel(x, src, dst, W0, al0, ar0, b0, W1, al1, ar1, b1, W2, al2, ar2, b2):
    from concourse.bass_utils import run_bass_kernel_spmd

    x = np.asarray(x, dtype=np.float32)
    g = _prep_graph(src, dst)
    CPH = g["CPH"]

    key = ("prog", CPH)
    if key not in _CACHE:
        _CACHE[key] = _build_program(CPH)
    nc = _CACHE[key]

    wc0 = _fold_w(np.asarray(W0, np.float32), np.asarray(al0, np.float32),
                  np.asarray(ar0, np.float32))
    wc1 = _fold_w(np.asarray(W1, np.float32), np.asarray(al1, np.float32),
                  np.asarray(ar1, np.float32))
    wc2 = _fold_w(np.asarray(W2, np.float32), np.asarray(al2, np.float32),
                  np.asarray(ar2, np.float32))

    iota128 = np.broadcast_to(np.arange(128, dtype=np.float32), (128, 128)) \
        .astype(BF16)
    iotaPb = np.arange(128, dtype=np.float32).reshape(128, 1).astype(BF16)
    ones128 = np.ones((128, 128), dtype=BF16)

    slot_g = g["slot_g"]
    in_maps = []
    for k in range(NC_N):
        lo, hi = k * NPC, (k + 1) * NPC
        xTk = np.zeros((128, SLOTS), dtype=BF16)
        xTk[:, slot_g[lo:hi]] = x[lo:hi].T.astype(BF16)
        in_maps.append({
            "xT": np.ascontiguousarray(xTk),
            "wc0": wc0, "wc1": wc1, "wc2": wc2,
            "idxA": g["idxA"][k], "idxB": g["idxB"][k],
            "dstlT": g["dstlT"][k], "dstlF": g["dstlF"][k],
            "iota128": iota128, "iotaPb": iotaPb,
            "ones128": ones128,
        })

    global _last_in_maps
    _last_in_maps = in_maps
    res = run_bass_kernel_spmd(nc, in_maps, core_ids=list(range(NC_N)))

    out = np.empty((N_NODES, 32), dtype=np.float32)
    for k in range(NC_N):
        lo, hi = k * NPC, (k + 1) * NPC
        out[lo:hi] = res.results[k]["out"][slot_g[lo:hi]]
    return out
